# revision 14
# baseline (speedup 1.0000x reference)
"""AttentiveFP (DGL) forward on 8 Trainium2 NeuronCores.

Sharding: graph-aligned node ranges per core (125 graphs each); edges sharded
by core of dst and sorted by dst. Segment ops via onehot matmuls over sorted
128-node tiles; softmax normalization deferred past the scatter. Layer-2
cross-core h[src] via AllGather of node-level pre-projected messages.
Big matmuls in fp32r.
"""
import numpy as np

P = 128
NCORES = 8
DG = 256


def _host_prep(node_feats, edge_feats, src, dst, graph_ids, params):
    B = 1000
    GPC = B // NCORES

    gid = graph_ids.astype(np.int64)
    node_start = np.searchsorted(gid, np.arange(0, B + 1, GPC))
    n_loc = np.diff(node_start)
    assert n_loc.min() > 0
    occ = np.bincount(gid, minlength=B)
    assert (occ > 0).all(), "empty graph not supported"

    core_of_node = np.minimum(gid // GPC, NCORES - 1)
    dst_core = core_of_node[dst]

    NT = int(np.ceil(n_loc.max() / P))
    NT = ((NT + 3) // 4) * 4
    N_SH = NT * P

    per_core = []
    K_CH = 1
    for c in range(NCORES):
        em = np.nonzero(dst_core == c)[0]
        dloc = dst[em] - node_start[c]
        order = np.argsort(dloc, kind="stable")
        em = em[order]
        dloc = dloc[order]
        cnt = np.bincount(dloc // P, minlength=NT)
        K_CH = max(K_CH, int(np.ceil(cnt.max() / P)))
        per_core.append((em, dloc, cnt))

    EC = NT * K_CH * P

    def slot(v):
        c = core_of_node[v]
        return c * N_SH + (v - node_start[c])

    f32 = np.float32

    p0 = params
    hv_new_np = node_feats @ np.asarray(p0["pn_W"], f32) + np.asarray(p0["pn_b"], f32)
    hv_new_np = np.where(hv_new_np > 0, hv_new_np, 0.01 * hv_new_np)
    q1_np = (hv_new_np @ np.asarray(p0["pe2_W"], f32)[0:DG, 0]
             + f32(p0["pe2_b"][0])).astype(f32)

    def prep_core(c):
        em, dloc, cnt = per_core[c]
        xeT = np.zeros((81, EC), f32)
        dst_sh = np.full((EC, 1), 999.0, f32)
        dst_li = np.zeros((EC, 1), np.int32)
        src_sl = np.zeros((EC, 1), np.int32)
        q1e = np.zeros((EC, 1), f32)
        pos_in_tile = np.concatenate([np.arange(n) for n in cnt])
        epos = (dloc // P) * (K_CH * P) + pos_in_tile
        xeT[0:64, epos] = node_feats[src[em]].T
        xeT[64:80, epos] = edge_feats[em].T
        xeT[80, epos] = 1.0
        dst_sh[epos, 0] = (dloc % P).astype(f32)
        dst_li[epos, 0] = dloc.astype(np.int32)
        src_sl[epos, 0] = slot(src[em]).astype(np.int32)
        q1e[epos, 0] = q1_np[dst[em]]

        nl = n_loc[c]
        hvT = np.zeros((65, N_SH), f32)
        hvT[0:64, :nl] = node_feats[node_start[c]:node_start[c + 1]].T
        hvT[64, :] = 1.0
        gid_sh = np.full((N_SH, 1), 999.0, f32)
        gid_sh[:nl, 0] = (gid[node_start[c]:node_start[c + 1]] - c * GPC).astype(f32)
        gid_li = np.zeros((N_SH, 1), np.int32)
        gid_li[:nl, 0] = (gid[node_start[c]:node_start[c + 1]] - c * GPC).astype(np.int32)
        nch = EC // P
        return dict(xeT=xeT,
                    dstsh_c=dst_sh.reshape(nch, P).T.copy(),
                    dstli_c=dst_li.reshape(nch, P).T.copy(),
                    srcsl_c=src_sl.reshape(nch, P).T.copy(),
                    q1e_c=q1e.reshape(nch, P).T.copy(),
                    hvT=hvT, gid_sh=gid_sh, gid_li=gid_li)

    def bc(v):
        return np.broadcast_to(np.asarray(v, f32)[None, :], (P, DG)).copy()

    p = params
    w = {}
    w["pn_W_aug"] = np.concatenate([p["pn_W"], p["pn_b"][None, :]], 0).astype(f32)
    w["pe1_W_aug"] = np.concatenate([p["pe1_W"], p["pe1_b"][None, :]], 0).astype(f32)
    Wmsg1 = np.zeros((DG, 258), f32)
    Wmsg1[:, 0:DG] = p["et_W"]
    Wmsg1[:, DG] = p["pe2_W"][DG:2 * DG, 0]
    w["Wmsg1"] = Wmsg1
    w["et_b_bc"] = bc(p["et_b"])
    for gi, g in (("1", "gru1"), ("2", "gru2")):
        Wih = np.asarray(p[f"{g}_Wih"], f32)
        Whh = np.asarray(p[f"{g}_Whh"], f32)
        bih = np.asarray(p[f"{g}_bih"], f32) - Wih.sum(1)  # elu(+1) fold
        bhh = np.asarray(p[f"{g}_bhh"], f32)
        w[f"W{gi}ihT"] = Wih.T.copy()
        w[f"W{gi}hhT"] = Whh.T.copy()
        w[f"br{gi}"] = (bih[0:DG] + bhh[0:DG]).reshape(2, P).T.copy()
        w[f"bz{gi}"] = (bih[DG:2 * DG] + bhh[DG:2 * DG]).reshape(2, P).T.copy()
        w[f"bin{gi}"] = bih[2 * DG:3 * DG].reshape(2, P).T.copy()
        w[f"bhn{gi}"] = bhh[2 * DG:3 * DG].reshape(2, P).T.copy()
    w["qdW"] = p["pe_W"][0:DG, 0:1].astype(f32)
    w["qsW"] = p["pe_W"][DG:2 * DG, 0:1].astype(f32)
    w["pnodeW"] = np.asarray(p["pnode_W"], f32)
    w["pnode_b_bc"] = bc(p["pnode_b"])
    for t in range(2):
        w[f"roTop{t}"] = p["ro_logit_W"][t][0:DG, 0:1].astype(f32)
        w[f"roBot{t}"] = p["ro_logit_W"][t][DG:2 * DG, 0:1].astype(f32)
        w[f"roProjW{t}"] = np.asarray(p["ro_proj_W"][t], f32)
        w[f"roProjB{t}"] = bc(p["ro_proj_b"][t])
        Wih = np.asarray(p["ro_gru_Wih"][t], f32)
        w[f"roIhT{t}"] = Wih.T.copy()
        w[f"roHhT{t}"] = np.asarray(p["ro_gru_Whh"][t], f32).T.copy()
        w[f"roBi{t}"] = (np.asarray(p["ro_gru_bih"][t], f32) - Wih.sum(1))[None, :].copy()
        w[f"roBh{t}"] = np.asarray(p["ro_gru_bhh"][t], f32)[None, :].copy()

    scal = dict(q1b=float(p["pe2_b"][0]), qdb=float(p["pe_b"][0]),
                rob=[float(p["ro_logit_b"][t][0]) for t in range(2)])
    in_maps = []
    for c in range(NCORES):
        m = prep_core(c)
        m.update(w)
        in_maps.append(m)
    return in_maps, dict(NT=NT, K_CH=K_CH, N_SH=N_SH, EC=EC, GPC=GPC, **scal)


def _build(cfg):
    import concourse.bass as bass
    import concourse.bacc as bacc
    import concourse.mybir as mybir
    import concourse.tile as tile
    from concourse.masks import make_identity

    F32 = mybir.dt.float32
    F32R = mybir.dt.float32r
    I32 = mybir.dt.int32
    AF = mybir.ActivationFunctionType
    OP = mybir.AluOpType

    NT, K_CH, N_SH = cfg["NT"], cfg["K_CH"], cfg["N_SH"]
    EC = cfg["EC"]
    NG = NT // 4

    nc = bacc.Bacc("TRN2", target_bir_lowering=False, debug=False, num_devices=NCORES)

    def din(name, shape, dt=None):
        return nc.dram_tensor(name, shape, dt or F32, kind="ExternalInput").ap()

    NCH = EC // P
    xeT_d = din("xeT", [81, EC], F32R)
    dstsh_d = din("dstsh_c", [P, NCH])
    dstli_d = din("dstli_c", [P, NCH], I32)
    srcsl_d = din("srcsl_c", [P, NCH], I32)
    q1e_d = din("q1e_c", [P, NCH])
    hvT_d = din("hvT", [65, N_SH], F32R)
    gid_sh_d = din("gid_sh", [N_SH, 1])
    gid_li_d = din("gid_li", [N_SH, 1], I32)
    pnW_d = din("pn_W_aug", [65, DG], F32R)
    pe1W_d = din("pe1_W_aug", [81, DG], F32R)
    Wmsg1_d = din("Wmsg1", [DG, 258], F32R)
    etb_d = din("et_b_bc", [P, DG])
    gwd = {}
    for gi in ("1", "2"):
        gwd[f"ihT{gi}"] = din(f"W{gi}ihT", [DG, 3 * DG], F32R)
        gwd[f"hhT{gi}"] = din(f"W{gi}hhT", [DG, 3 * DG], F32R)
        for b in ("br", "bz", "bin", "bhn"):
            gwd[f"{b}{gi}"] = din(f"{b}{gi}", [P, 2])
    qdW_d = din("qdW", [DG, 1], F32R)
    qsW_d = din("qsW", [DG, 1], F32R)
    pnodeW_d = din("pnodeW", [DG, DG], F32R)
    pnb_d = din("pnode_b_bc", [P, DG])
    rod = {}
    for t in range(2):
        rod[f"top{t}"] = din(f"roTop{t}", [DG, 1], F32R)
        rod[f"bot{t}"] = din(f"roBot{t}", [DG, 1], F32R)
        rod[f"projW{t}"] = din(f"roProjW{t}", [DG, DG], F32R)
        rod[f"projB{t}"] = din(f"roProjB{t}", [P, DG])
        rod[f"ihT{t}"] = din(f"roIhT{t}", [DG, 3 * DG], F32R)
        rod[f"hhT{t}"] = din(f"roHhT{t}", [DG, 3 * DG], F32R)
        rod[f"bi{t}"] = din(f"roBi{t}", [1, 3 * DG], F32R)
        rod[f"bh{t}"] = din(f"roBh{t}", [1, 3 * DG], F32R)

    out_d = nc.dram_tensor("o", [P, DG], F32, kind="ExternalOutput").ap()

    hvnT_dram = nc.dram_tensor("hvnT", [DG, N_SH], F32R).ap()
    h1T_dram = nc.dram_tensor("h1T", [DG, N_SH], F32R).ap()
    h2T_dram = nc.dram_tensor("h2T", [DG, N_SH], F32R).ap()
    ctxT_dram = nc.dram_tensor("ctxT", [DG, N_SH], F32R).ap()
    q1_dram = nc.dram_tensor("q1d", [N_SH, 1], F32).ap()
    qd_dram = nc.dram_tensor("qdd", [N_SH, 1], F32).ap()
    cc_in = nc.dram_tensor("cc_in", [N_SH, 258], F32).ap()
    cc_out = nc.dram_tensor("cc_out", [NCORES * N_SH, 258], F32,
                            addr_space="Shared").ap()

    with tile.TileContext(nc) as tc:
        _cms = []

        def enter_pool(**kw):
            cm = tc.tile_pool(**kw)
            _cms.append(cm)
            return cm, cm.__enter__()

        _, cp = enter_pool(name="consts", bufs=1)
        _, sp = enter_pool(name="w2", bufs=2)
        _, sp1 = enter_pool(name="w1", bufs=1)
        _, sp3 = enter_pool(name="w3", bufs=2)
        _, sp4 = enter_pool(name="w4", bufs=2)

        iota_f = cp.tile([P, P], F32)
        nc.gpsimd.iota(iota_f[:], pattern=[[1, P]], base=0, channel_multiplier=0,
                       allow_small_or_imprecise_dtypes=True)
        ident = cp.tile([P, P], F32)
        make_identity(nc, ident[:])
        identr = cp.tile([P, P], F32R)
        nc.vector.tensor_copy(out=identr[:], in_=ident[:])
        ones_f = cp.tile([1, P], F32)
        nc.vector.memset(ones_f[:], 1.0)
        ones_row = cp.tile([1, P], F32R)
        nc.vector.tensor_copy(out=ones_row[:], in_=ones_f[:])

        _ctr = [0]

        def _utag():
            _ctr[0] += 1
            return f"c{_ctr[0]}"

        def load(ap_d, shape, dt=F32, pool=None, tag=None):
            t = (pool or cp).tile(shape, dt, tag=tag or _utag())
            nc.sync.dma_start(out=t[:], in_=ap_d[:])
            return t

        def load2(ap_d, w, dt=F32R, pool=None, tags=None):
            out = []
            for h in range(2):
                t = (pool or cp).tile([P, w], dt, tag=tags[h] if tags else _utag())
                nc.sync.dma_start(out=t[:], in_=ap_d[h * P:(h + 1) * P, :])
                out.append(t)
            return out

        pnW = load(pnW_d, [65, DG], F32R)
        pe1W = load(pe1W_d, [81, DG], F32R)
        Wmsg1 = load2(Wmsg1_d, 258)
        etb = load(etb_d, [P, DG])
        qdW = load2(qdW_d, 1)
        qsW = load2(qsW_d, 1)
        pnodeW = load2(pnodeW_d, DG)
        pnb = load(pnb_d, [P, DG])
        GB = {}
        for gi in ("1", "2"):
            for b in ("br", "bz", "bin", "bhn"):
                GB[f"{b}{gi}"] = load(gwd[f"{b}{gi}"], [P, 2])

        # ---------- helpers ----------
        def safe_recip(s_ap):
            ind = sp4.tile([P, 1], F32, tag="ind")
            nc.vector.tensor_scalar(out=ind[:], in0=s_ap, scalar1=0.0,
                                    scalar2=None, op0=OP.is_gt)
            ssafe = sp4.tile([P, 1], F32, tag="ssafe")
            nc.vector.tensor_scalar(out=ssafe[:], in0=ind[:], scalar1=-1.0,
                                    scalar2=1.0, op0=OP.mult, op1=OP.add)
            nc.vector.tensor_tensor(out=ssafe[:], in0=ssafe[:], in1=s_ap, op=OP.add)
            inv = sp4.tile([P, 1], F32, tag="inv")
            nc.vector.reciprocal(out=inv[:], in_=ssafe[:])
            return ind, inv

        def elu1(src_ap, out_tag):
            mn = sp3.tile([P, DG], F32, tag="mn")
            nc.vector.tensor_scalar(out=mn[:], in0=src_ap, scalar1=0.0, scalar2=None,
                                    op0=OP.min)
            ex = sp3.tile([P, DG], F32, tag="ex")
            nc.scalar.activation(out=ex[:], in_=mn[:], func=AF.Exp)
            rl = sp3.tile([P, DG], F32, tag=out_tag)
            nc.scalar.activation(out=rl[:], in_=src_ap, func=AF.Relu)
            nc.vector.tensor_tensor(out=rl[:], in0=rl[:], in1=ex[:], op=OP.add)
            return rl

        def seg_post(ctx_ps, ppt, bias_bc, nt):
            ind, inv = safe_recip(ctx_ps[:, 256:257])
            cn = sp3.tile([P, DG], F32, tag="cn")
            nc.vector.tensor_scalar(out=cn[:], in0=ctx_ps[:, 0:DG], scalar1=inv[:, :1],
                                    scalar2=None, op0=OP.mult)
            tb = sp3.tile([P, DG], F32, tag="tb")
            nc.vector.tensor_scalar(out=tb[:], in0=bias_bc[:], scalar1=ind[:, :1],
                                    scalar2=None, op0=OP.mult)
            nc.vector.tensor_tensor(out=cn[:], in0=cn[:], in1=tb[:], op=OP.add)
            rl = elu1(cn[:], "rlp")
            for h in range(2):
                tp = ppt.tile([P, P], F32, tag="tpost")
                nc.tensor.transpose(out=tp[:], in_=rl[:, h * P:(h + 1) * P],
                                    identity=ident[:])
                cc = sp4.tile([P, P], F32R, tag="ctxc")
                nc.vector.tensor_copy(out=cc[:], in_=tp[:])
                nc.sync.dma_start(out=ctxT_dram[h * P:(h + 1) * P, nt * P:(nt + 1) * P],
                                  in_=cc[:])

        def edge_phase(layer, pp2, pp1):
            for nt in range(NT):
                ctx_ps = pp2.tile([P, 258], F32, tag="ctx_ps")
                nch = 0
                rem = K_CH
                while rem > 0:
                    take = min(4, rem)
                    base = (nt * K_CH + nch) * P
                    W = take * P
                    if layer == 1:
                        xe = sp.tile([81, 512], F32R, tag="xe")
                        nc.sync.dma_start(out=xe[:, 0:W], in_=xeT_d[:, base:base + W])
                        he_ps, he = [], []
                        for h in range(2):
                            hp = pp1.tile([P, 512], F32, tag=f"he{h}")
                            nc.tensor.matmul(out=hp[:, 0:W],
                                             lhsT=pe1W[:, h * P:(h + 1) * P],
                                             rhs=xe[:, 0:W], start=True, stop=True)
                            hs = sp.tile([P, 512], F32R, tag=f"hes{h}")
                            nc.scalar.activation(out=hs[:, 0:W], in_=hp[:, 0:W],
                                                 func=AF.Lrelu, alpha=0.01)
                            he_ps.append(hp)
                            he.append(hs)
                    cb = nt * K_CH + nch
                    dstf4 = sp4.tile([P, 4], F32, tag="dstf")
                    nc.sync.dma_start(out=dstf4[:, 0:take], in_=dstsh_d[:, cb:cb + take])
                    if layer == 1:
                        q1e4 = sp4.tile([P, 4], F32, tag="q1e")
                        nc.sync.dma_start(out=q1e4[:, 0:take], in_=q1e_d[:, cb:cb + take])
                    else:
                        dsti4 = sp4.tile([P, 4], I32, tag="dsti")
                        nc.sync.dma_start(out=dsti4[:, 0:take], in_=dstli_d[:, cb:cb + take])
                        srci4 = sp4.tile([P, 4], I32, tag="srci")
                        nc.sync.dma_start(out=srci4[:, 0:take], in_=srcsl_d[:, cb:cb + take])
                    for s in range(take):
                        eb = base + s * P
                        if layer == 1:
                            qg = q1e4[:, s:s + 1]
                            mg_ps = pp2.tile([P, 258], F32, tag="mg_ps")
                            for h in range(2):
                                nc.tensor.matmul(out=mg_ps[:],
                                                 lhsT=he[h][:, s * P:(s + 1) * P],
                                                 rhs=Wmsg1[h][:], start=(h == 0),
                                                 stop=(h == 1))
                            logit_src = mg_ps[:, 256:257]
                            val_src = mg_ps[:, 0:DG]
                        else:
                            gath = sp4.tile([P, 258], F32, tag="gath")
                            nc.gpsimd.indirect_dma_start(
                                out=gath[:], out_offset=None, in_=cc_out[:],
                                in_offset=bass.IndirectOffsetOnAxis(ap=srci4[:, s:s + 1], axis=0))
                            qgt = sp4.tile([P, 1], F32, tag="qg")
                            nc.gpsimd.indirect_dma_start(
                                out=qgt[:], out_offset=None, in_=qd_dram[:],
                                in_offset=bass.IndirectOffsetOnAxis(ap=dsti4[:, s:s + 1], axis=0))
                            qg = qgt
                            logit_src = gath[:, 256:257]
                            val_src = gath[:, 0:DG]
                        lr = sp4.tile([P, 1], F32, tag="lr")
                        nc.scalar.activation(out=lr[:], in_=logit_src, func=AF.Lrelu,
                                             bias=qg[:, 0:1], alpha=0.01)
                        ee = sp4.tile([P, 1], F32, tag="ee")
                        nc.scalar.activation(out=ee[:], in_=lr[:], func=AF.Exp)
                        msg = sp4.tile([P, 258], F32R, tag="msg")
                        nc.vector.tensor_scalar(out=msg[:, 0:DG], in0=val_src,
                                                scalar1=ee[:, :1], scalar2=None,
                                                op0=OP.mult)
                        nc.vector.tensor_copy(out=msg[:, 256:257], in_=ee[:])
                        nc.vector.tensor_scalar(out=msg[:, 257:258], in0=ee[:],
                                                scalar1=0.0, scalar2=None, op0=OP.mult)
                        oh = sp4.tile([P, P], F32R, tag="oh")
                        nc.vector.tensor_scalar(out=oh[:], in0=iota_f[:],
                                                scalar1=dstf4[:, s:s + 1], scalar2=None,
                                                op0=OP.is_equal)
                        nc.tensor.matmul(out=ctx_ps[:], lhsT=oh[:], rhs=msg[:],
                                         start=(nch == 0 and s == 0),
                                         stop=(rem == take and s == take - 1))
                    nch += take
                    rem -= take
                seg_post(ctx_ps, pp1, etb if layer == 1 else pnb, nt)

        def gru_phase(gi, h_dram, ppg, ppm, post_group):
            """GRU over 512-node groups; x from ctxT_dram, h from h_dram.
            post_group(g, h_out_tiles) emits per-group extras."""
            ihT = load2(gwd[f"ihT{gi}"], 3 * DG, pool=sp1, tags=("wihA", "wihB"))
            hhT = load2(gwd[f"hhT{gi}"], 3 * DG, pool=sp1, tags=("whhA", "whhB"))
            br, bz = GB[f"br{gi}"], GB[f"bz{gi}"]
            bin_, bhn = GB[f"bin{gi}"], GB[f"bhn{gi}"]
            for g in range(NG):
                c0 = g * 512
                hv = []
                for h in range(2):
                    t = sp.tile([P, 512], F32R, tag=f"hv{h}")
                    nc.sync.dma_start(out=t[:], in_=h_dram[h * P:(h + 1) * P, c0:c0 + 512])
                    hv.append(t)
                x = []
                for h in range(2):
                    t = sp.tile([P, 512], F32R, tag=f"xc{h}")
                    nc.sync.dma_start(out=t[:], in_=ctxT_dram[h * P:(h + 1) * P, c0:c0 + 512])
                    x.append(t)
                hout = []
                for jh in range(2):
                    def mk(jofs, sides, tag):
                        ps = ppg.tile([P, 512], F32, tag=tag)
                        n = 0
                        tot = 2 * len(sides)
                        for (W2, rhs2) in sides:
                            for k in range(2):
                                nc.tensor.matmul(
                                    out=ps[:],
                                    lhsT=W2[k][:, jofs + jh * P:jofs + (jh + 1) * P],
                                    rhs=rhs2[k][:], start=(n == 0), stop=(n == tot - 1))
                                n += 1
                        return ps
                    p_r = mk(0, [(ihT, x), (hhT, hv)], "gps_r")
                    p_z = mk(DG, [(ihT, x), (hhT, hv)], "gps_z")
                    p_i = mk(2 * DG, [(ihT, x)], "gps_i")
                    p_h = mk(2 * DG, [(hhT, hv)], "gps_h")
                    r = sp3.tile([P, 512], F32, tag="r")
                    nc.scalar.activation(out=r[:], in_=p_r[:], func=AF.Sigmoid,
                                         bias=br[:, jh:jh + 1])
                    z = sp3.tile([P, 512], F32, tag="z")
                    nc.scalar.activation(out=z[:], in_=p_z[:], func=AF.Sigmoid,
                                         bias=bz[:, jh:jh + 1])
                    t1 = sp3.tile([P, 512], F32, tag="t1")
                    nc.vector.tensor_scalar(out=t1[:], in0=p_h[:], scalar1=bhn[:, jh:jh + 1],
                                            scalar2=None, op0=OP.add)
                    nc.vector.tensor_tensor(out=t1[:], in0=r[:], in1=t1[:], op=OP.mult)
                    nc.vector.tensor_tensor(out=t1[:], in0=p_i[:], in1=t1[:], op=OP.add)
                    nn = sp3.tile([P, 512], F32, tag="nn")
                    nc.scalar.activation(out=nn[:], in_=t1[:], func=AF.Tanh,
                                         bias=bin_[:, jh:jh + 1])
                    d = sp3.tile([P, 512], F32, tag="d")
                    nc.vector.tensor_tensor(out=d[:], in0=hv[jh][:], in1=nn[:],
                                            op=OP.subtract)
                    nc.vector.tensor_tensor(out=d[:], in0=z[:], in1=d[:], op=OP.mult)
                    nc.vector.tensor_tensor(out=d[:], in0=nn[:], in1=d[:], op=OP.add)
                    ho = sp.tile([P, 512], F32R, tag=f"ho{jh}")
                    nc.scalar.activation(out=ho[:], in_=d[:], func=AF.Relu)
                    hout.append(ho)
                post_group(g, hout, ppm)

        # ---------- Phase A ----------
        ppA_cm, ppA = enter_pool(name="ppA", bufs=2, space="PSUM")
        for g in range(NG):
            c0 = g * 512
            hvt = sp.tile([65, 512], F32R, tag="hvt")
            nc.sync.dma_start(out=hvt[:], in_=hvT_d[:, c0:c0 + 512])
            hvn = []
            for h in range(2):
                ps = ppA.tile([P, 512], F32, tag="Aps")
                nc.tensor.matmul(out=ps[:], lhsT=pnW[:, h * P:(h + 1) * P], rhs=hvt[:],
                                 start=True, stop=True)
                hm = sp.tile([P, 512], F32R, tag=f"hvn{h}")
                nc.scalar.activation(out=hm[:], in_=ps[:], func=AF.Lrelu, alpha=0.01)
                nc.sync.dma_start(out=hvnT_dram[h * P:(h + 1) * P, c0:c0 + 512], in_=hm[:])
                hvn.append(hm)
        ppA_cm.__exit__(None, None, None)

        # ---------- Phase B ----------
        pB2_cm, pB2 = enter_pool(name="pB2", bufs=2, space="PSUM")
        pB1_cm, pB1 = enter_pool(name="pB1", bufs=1, space="PSUM")
        edge_phase(1, pB2, pB1)
        pB1_cm.__exit__(None, None, None)
        pB2_cm.__exit__(None, None, None)

        # ---------- Phase C: GRU1 -> h1, pre_msg, qd, qs ----------
        pC1_cm, pC1 = enter_pool(name="pC1", bufs=1, space="PSUM")
        pC2_cm, pC2 = enter_pool(name="pC2", bufs=2, space="PSUM")

        def c_post(g, hout, ppm):
            c0 = g * 512
            for jh in range(2):
                nc.sync.dma_start(out=h1T_dram[jh * P:(jh + 1) * P, c0:c0 + 512],
                                  in_=hout[jh][:])
            for s in range(4):
                pm = ppm.tile([P, DG], F32, tag="pm")
                for k in range(2):
                    nc.tensor.matmul(out=pm[:], lhsT=hout[k][:, s * P:(s + 1) * P],
                                     rhs=pnodeW[k][:], start=(k == 0), stop=(k == 1))
                pms = sp4.tile([P, DG], F32, tag="pms")
                nc.vector.tensor_copy(out=pms[:], in_=pm[:])
                nc.sync.dma_start(out=cc_in[c0 + s * P:c0 + (s + 1) * P, 0:DG], in_=pms[:])
            for (Wv, dstap, bias, tg) in (
                    (qdW, qd_dram[c0:c0 + 512, 0:1], cfg["qdb"], "qrow"),
                    (qsW, cc_in[c0:c0 + 512, 256:257], 0.0, "qrow")):
                qp = ppm.tile([1, 512], F32, tag=tg)
                for k in range(2):
                    nc.tensor.matmul(out=qp[:], lhsT=Wv[k][:], rhs=hout[k][:],
                                     start=(k == 0), stop=(k == 1))
                qs = sp4.tile([1, 512], F32, tag="qrs")
                nc.scalar.activation(out=qs[:], in_=qp[:], func=AF.Copy, bias=bias)
                nc.sync.dma_start(out=dstap.rearrange("a b -> b a"), in_=qs[:])

        gru_phase("1", hvnT_dram, pC1, pC2, c_post)
        pC2_cm.__exit__(None, None, None)
        pC1_cm.__exit__(None, None, None)

        # ---------- Phase D: AllGather ----------
        nc.gpsimd.collective_compute(
            "AllGather", mybir.AluOpType.bypass,
            replica_groups=[list(range(NCORES))],
            ins=[cc_in[:]], outs=[cc_out[:]])

        # ---------- Phase E ----------
        pE2_cm, pE2 = enter_pool(name="pE2", bufs=2, space="PSUM")
        pE1_cm, pE1 = enter_pool(name="pE1", bufs=1, space="PSUM")
        edge_phase(2, pE2, pE1)
        pE1_cm.__exit__(None, None, None)
        pE2_cm.__exit__(None, None, None)

        # ---------- Phase F: GRU2 -> h2, g0 ----------
        pF1_cm, pF1 = enter_pool(name="pF1", bufs=1, space="PSUM")
        pF2_cm, pF2 = enter_pool(name="pF2", bufs=2, space="PSUM")
        g0_ps = pF1.tile([P, DG], F32, tag="g0ps")
        nmm = [0]

        def f_post(g, hout, ppm):
            c0 = g * 512
            for jh in range(2):
                nc.sync.dma_start(out=h2T_dram[jh * P:(jh + 1) * P, c0:c0 + 512],
                                  in_=hout[jh][:])
            for s in range(4):
                nt = g * 4 + s
                h2n = sp4.tile([P, DG], F32R, tag="h2n")
                for h in range(2):
                    tp = ppm.tile([P, P], F32R, tag="ftp")
                    nc.tensor.transpose(out=tp[:], in_=hout[h][:, s * P:(s + 1) * P],
                                        identity=identr[:])
                    nc.vector.tensor_copy(out=h2n[:, h * P:(h + 1) * P], in_=tp[:])
                gidf = sp4.tile([P, 1], F32, tag="gidf")
                nc.sync.dma_start(out=gidf[:], in_=gid_sh_d[nt * P:(nt + 1) * P, :])
                ohg = sp4.tile([P, P], F32R, tag="ohg")
                nc.vector.tensor_scalar(out=ohg[:], in0=iota_f[:], scalar1=gidf[:, :1],
                                        scalar2=None, op0=OP.is_equal)
                nc.tensor.matmul(out=g0_ps[:], lhsT=ohg[:], rhs=h2n[:],
                                 start=(nmm[0] == 0), stop=(nmm[0] == NT - 1))
                nmm[0] += 1

        gru_phase("2", h1T_dram, pF1, pF2, f_post)
        g0sb = cp.tile([P, DG], F32)
        nc.vector.tensor_copy(out=g0sb[:], in_=g0_ps[:])
        pF2_cm.__exit__(None, None, None)
        pF1_cm.__exit__(None, None, None)

        # ---------- Phase G: readout ----------
        pG1_cm, pG1 = enter_pool(name="pG1", bufs=1, space="PSUM")
        pG2_cm, pG2 = enter_pool(name="pG2", bufs=2, space="PSUM")

        def transpose2(src_ap_fn, dst_tiles):
            for h in range(2):
                tp = pG1.tile([P, P], F32, tag="gtp1")
                nc.tensor.transpose(out=tp[:], in_=src_ap_fn(h), identity=ident[:])
                nc.vector.tensor_copy(out=dst_tiles[h][:], in_=tp[:])

        g_cur = g0sb
        for t in range(2):
            roTop = load2(rod[f"top{t}"], 1, pool=sp1, tags=("rotA", "rotB"))
            roBot = load2(rod[f"bot{t}"], 1, pool=sp1, tags=("robA", "robB"))
            roPW = load2(rod[f"projW{t}"], DG, pool=sp1, tags=("ropA", "ropB"))
            roPB = load(rod[f"projB{t}"], [P, DG], pool=sp1, tag="ropb")
            roIh = load2(rod[f"ihT{t}"], 3 * DG, pool=sp1, tags=("wihA", "wihB"))
            roHh = load2(rod[f"hhT{t}"], 3 * DG, pool=sp1, tags=("whhA", "whhB"))
            roBi = load(rod[f"bi{t}"], [1, 3 * DG], F32R, pool=sp1, tag="robi")
            roBh = load(rod[f"bh{t}"], [1, 3 * DG], F32R, pool=sp1, tag="robh")

            rg = sp3.tile([P, DG], F32, tag="rg")
            nc.scalar.activation(out=rg[:], in_=g_cur[:], func=AF.Relu)
            rgT = [sp4.tile([P, P], F32R, tag=f"rgT{h}", name=f"rgT{h}") for h in range(2)]
            transpose2(lambda h: rg[:, h * P:(h + 1) * P], rgT)
            qgp = pG1.tile([1, P], F32, tag="smallp")
            for h in range(2):
                nc.tensor.matmul(out=qgp[:], lhsT=roTop[h][:], rhs=rgT[h][:],
                                 start=(h == 0), stop=(h == 1))
            qgs = sp4.tile([1, P], F32, tag="qgs")
            nc.scalar.activation(out=qgs[:], in_=qgp[:], func=AF.Copy, bias=cfg["rob"][t])
            nc.sync.dma_start(out=q1_dram[0:P, 0:1].rearrange("a b -> b a"), in_=qgs[:])

            wg_ps = pG1.tile([P, 258], F32, tag="wgps")
            for g in range(NG):
                c0 = g * 512
                h2g = []
                for h in range(2):
                    tt = sp.tile([P, 512], F32R, tag=f"hv{h}")
                    nc.sync.dma_start(out=tt[:], in_=h2T_dram[h * P:(h + 1) * P, c0:c0 + 512])
                    h2g.append(tt)
                qhp = pG1.tile([1, 512], F32, tag="smallp")
                for h in range(2):
                    nc.tensor.matmul(out=qhp[:], lhsT=roBot[h][:], rhs=h2g[h][:],
                                     start=(h == 0), stop=(h == 1))
                qhs = sp4.tile([1, 512], F32, tag="qhs")
                nc.vector.tensor_copy(out=qhs[:], in_=qhp[:])
                for s in range(4):
                    nt = g * 4 + s
                    qtp = pG1.tile([P, 1], F32, tag="qtp")
                    nc.tensor.transpose(out=qtp[:], in_=qhs[0:1, s * P:(s + 1) * P],
                                        identity=ident[0:1, 0:1])
                    gidi = sp4.tile([P, 1], I32, tag="gidi")
                    nc.sync.dma_start(out=gidi[:], in_=gid_li_d[nt * P:(nt + 1) * P, :])
                    qgg = sp4.tile([P, 1], F32, tag="qgg")
                    nc.gpsimd.indirect_dma_start(
                        out=qgg[:], out_offset=None, in_=q1_dram[:],
                        in_offset=bass.IndirectOffsetOnAxis(ap=gidi[:, :1], axis=0))
                    lr = sp4.tile([P, 1], F32, tag="lr")
                    nc.scalar.activation(out=lr[:], in_=qtp[:], func=AF.Lrelu,
                                         bias=qgg[:, :1], alpha=0.01)
                    ee = sp4.tile([P, 1], F32, tag="ee")
                    nc.scalar.activation(out=ee[:], in_=lr[:], func=AF.Exp)
                    h2n = sp4.tile([P, DG], F32, tag="h2nG")
                    for h in range(2):
                        tp = pG2.tile([P, P], F32R, tag="gtpr")
                        nc.tensor.transpose(out=tp[:], in_=h2g[h][:, s * P:(s + 1) * P],
                                            identity=identr[:])
                        nc.vector.tensor_copy(out=h2n[:, h * P:(h + 1) * P], in_=tp[:])
                    hw = sp4.tile([P, 258], F32R, tag="hw")
                    nc.vector.tensor_scalar(out=hw[:, 0:DG], in0=h2n[:], scalar1=ee[:, :1],
                                            scalar2=None, op0=OP.mult)
                    nc.vector.tensor_copy(out=hw[:, 256:257], in_=ee[:])
                    nc.vector.tensor_scalar(out=hw[:, 257:258], in0=ee[:],
                                            scalar1=0.0, scalar2=None, op0=OP.mult)
                    gidf = sp4.tile([P, 1], F32, tag="gidf")
                    nc.sync.dma_start(out=gidf[:], in_=gid_sh_d[nt * P:(nt + 1) * P, :])
                    ohg = sp4.tile([P, P], F32R, tag="ohg")
                    nc.vector.tensor_scalar(out=ohg[:], in0=iota_f[:], scalar1=gidf[:, :1],
                                            scalar2=None, op0=OP.is_equal)
                    nc.tensor.matmul(out=wg_ps[:], lhsT=ohg[:], rhs=hw[:],
                                     start=(nt == 0), stop=(nt == NT - 1))
            ind, inv = safe_recip(wg_ps[:, 256:257])
            wgn = sp3.tile([P, DG], F32, tag="wgn")
            nc.vector.tensor_scalar(out=wgn[:], in0=wg_ps[:, 0:DG], scalar1=inv[:, :1],
                                    scalar2=None, op0=OP.mult)
            wgnT = [sp4.tile([P, P], F32R, tag=f"wgnT{h}", name=f"wgnT{h}") for h in range(2)]
            transpose2(lambda h: wgn[:, h * P:(h + 1) * P], wgnT)
            prj = pG1.tile([P, DG], F32, tag="smallp")
            for h in range(2):
                nc.tensor.matmul(out=prj[:], lhsT=wgnT[h][:], rhs=roPW[h][:],
                                 start=(h == 0), stop=(h == 1))
            tb = sp3.tile([P, DG], F32, tag="tb")
            nc.vector.tensor_scalar(out=tb[:], in0=roPB[:], scalar1=ind[:, :1],
                                    scalar2=None, op0=OP.mult)
            grp = sp3.tile([P, DG], F32, tag="grp")
            nc.vector.tensor_tensor(out=grp[:], in0=prj[:], in1=tb[:], op=OP.add)
            cfed = elu1(grp[:], "cfed")
            cT = [sp4.tile([P, P], F32R, tag=f"cT{h}", name=f"cT{h}") for h in range(2)]
            gT = [sp4.tile([P, P], F32R, tag=f"gT{h}", name=f"gT{h}") for h in range(2)]
            transpose2(lambda h: cfed[:, h * P:(h + 1) * P], cT)
            transpose2(lambda h: g_cur[:, h * P:(h + 1) * P], gT)

            def ro_gate(jofs, xT, WT, brow, tag):
                ps = pG2.tile([P, DG], F32, tag=tag)
                for k in range(2):
                    nc.tensor.matmul(out=ps[:], lhsT=xT[k][:],
                                     rhs=WT[k][:, jofs:jofs + DG],
                                     start=(k == 0), stop=False)
                nc.tensor.matmul(out=ps[:], lhsT=ones_row[:],
                                 rhs=brow[:, jofs:jofs + DG], start=False, stop=True)
                return ps

            ph_sb = {}
            for (nm, jofs) in (("r", 0), ("z", DG), ("n", 2 * DG)):
                ps = ro_gate(jofs, gT, roHh, roBh, "rops")
                sb = sp3.tile([P, DG], F32, tag=f"ph{nm}")
                nc.vector.tensor_copy(out=sb[:], in_=ps[:])
                ph_sb[nm] = sb
            r = sp3.tile([P, DG], F32, tag="ror")
            ps = ro_gate(0, cT, roIh, roBi, "rops")
            nc.vector.tensor_tensor(out=r[:], in0=ps[:], in1=ph_sb["r"][:], op=OP.add)
            nc.scalar.activation(out=r[:], in_=r[:], func=AF.Sigmoid)
            z = sp3.tile([P, DG], F32, tag="roz")
            ps = ro_gate(DG, cT, roIh, roBi, "rops")
            nc.vector.tensor_tensor(out=z[:], in0=ps[:], in1=ph_sb["z"][:], op=OP.add)
            nc.scalar.activation(out=z[:], in_=z[:], func=AF.Sigmoid)
            ps = ro_gate(2 * DG, cT, roIh, roBi, "rops")
            nc.vector.tensor_tensor(out=ph_sb["n"][:], in0=r[:], in1=ph_sb["n"][:],
                                    op=OP.mult)
            nc.vector.tensor_tensor(out=ph_sb["n"][:], in0=ps[:], in1=ph_sb["n"][:],
                                    op=OP.add)
            nn = sp3.tile([P, DG], F32, tag="ronn")
            nc.scalar.activation(out=nn[:], in_=ph_sb["n"][:], func=AF.Tanh)
            gnew = cp.tile([P, DG], F32, tag=f"gnew{t}")
            nc.vector.tensor_tensor(out=gnew[:], in0=g_cur[:], in1=nn[:], op=OP.subtract)
            nc.vector.tensor_tensor(out=gnew[:], in0=z[:], in1=gnew[:], op=OP.mult)
            nc.vector.tensor_tensor(out=gnew[:], in0=nn[:], in1=gnew[:], op=OP.add)
            g_cur = gnew
        nc.sync.dma_start(out=out_d[:], in_=g_cur[:])
        pG2_cm.__exit__(None, None, None)
        pG1_cm.__exit__(None, None, None)

        for cm in reversed(_cms[:5]):
            cm.__exit__(None, None, None)

    nc.compile()
    return nc


def kernel(node_feats, edge_feats, src, dst, graph_ids, params):
    from concourse import bass_utils
    node_feats = np.asarray(node_feats, np.float32)
    edge_feats = np.asarray(edge_feats, np.float32)
    src = np.asarray(src).astype(np.int64)
    dst = np.asarray(dst).astype(np.int64)
    graph_ids = np.asarray(graph_ids).astype(np.int64)
    params = {k: np.asarray(v) for k, v in params.items()}

    in_maps, cfg = _host_prep(node_feats, edge_feats, src, dst, graph_ids, params)
    nc = _build(cfg)
    res = bass_utils.run_bass_kernel_spmd(nc, in_maps, core_ids=list(range(NCORES)))
    out = np.concatenate([res.results[c]["o"][:cfg["GPC"]] for c in range(NCORES)], 0)
    return out.astype(np.float32)


# revision 22
# speedup vs baseline: 1.0753x; 1.0753x over previous
"""AttentiveFP (DGL) forward on 8 Trainium2 NeuronCores.

Sharding: graph-aligned node ranges per core (125 graphs each); edges sharded
by core of dst and sorted by dst. Segment ops via onehot matmuls over sorted
128-node tiles; softmax normalization deferred past the scatter. Layer-2
cross-core h[src] via AllGather of node-level pre-projected messages.
Big matmuls in fp32r.
"""
import numpy as np

P = 128
NCORES = 8
DG = 256


def _host_prep(node_feats, edge_feats, src, dst, graph_ids, params):
    B = 1000
    GPC = B // NCORES

    gid = graph_ids.astype(np.int64)
    node_start = np.searchsorted(gid, np.arange(0, B + 1, GPC))
    n_loc = np.diff(node_start)
    assert n_loc.min() > 0
    occ = np.bincount(gid, minlength=B)
    assert (occ > 0).all(), "empty graph not supported"

    core_of_node = np.minimum(gid // GPC, NCORES - 1)
    dst_core = core_of_node[dst]

    NT = int(np.ceil(n_loc.max() / P))
    NT = ((NT + 3) // 4) * 4
    N_SH = NT * P

    per_core = []
    K_CH = 1
    for c in range(NCORES):
        em = np.nonzero(dst_core == c)[0]
        dloc = dst[em] - node_start[c]
        order = np.argsort(dloc, kind="stable")
        em = em[order]
        dloc = dloc[order]
        cnt = np.bincount(dloc // P, minlength=NT)
        K_CH = max(K_CH, int(np.ceil(cnt.max() / P)))
        per_core.append((em, dloc, cnt))

    EC = NT * K_CH * P

    def slot(v):
        c = core_of_node[v]
        return c * N_SH + (v - node_start[c])

    f32 = np.float32

    p0 = params
    hv_new_np = node_feats @ np.asarray(p0["pn_W"], f32) + np.asarray(p0["pn_b"], f32)
    hv_new_np = np.where(hv_new_np > 0, hv_new_np, 0.01 * hv_new_np)
    q1_np = (hv_new_np @ np.asarray(p0["pe2_W"], f32)[0:DG, 0]
             + f32(p0["pe2_b"][0])).astype(f32)

    def prep_core(c):
        em, dloc, cnt = per_core[c]
        xeT = np.zeros((81, EC), f32)
        dst_sh = np.full((EC, 1), 999.0, f32)
        dst_li = np.zeros((EC, 1), np.int32)
        src_sl = np.zeros((EC, 1), np.int32)
        q1e = np.zeros((EC, 1), f32)
        pos_in_tile = np.concatenate([np.arange(n) for n in cnt])
        epos = (dloc // P) * (K_CH * P) + pos_in_tile
        xeT[0:64, epos] = node_feats[src[em]].T
        xeT[64:80, epos] = edge_feats[em].T
        xeT[80, epos] = 1.0
        dst_sh[epos, 0] = (dloc % P).astype(f32)
        dst_li[epos, 0] = dloc.astype(np.int32)
        src_sl[epos, 0] = slot(src[em]).astype(np.int32)
        q1e[epos, 0] = q1_np[dst[em]]

        nl = n_loc[c]
        hvT = np.zeros((65, N_SH), f32)
        hvT[0:64, :nl] = node_feats[node_start[c]:node_start[c + 1]].T
        hvT[64, :] = 1.0
        gid_sh = np.full((N_SH, 1), 999.0, f32)
        gid_sh[:nl, 0] = (gid[node_start[c]:node_start[c + 1]] - c * GPC).astype(f32)
        gid_li = np.zeros((N_SH, 1), np.int32)
        gid_li[:nl, 0] = (gid[node_start[c]:node_start[c + 1]] - c * GPC).astype(np.int32)
        nch = EC // P
        return dict(xeT=xeT,
                    dstsh_c=dst_sh.reshape(nch, P).T.copy(),
                    dstli_c=dst_li.reshape(nch, P).T.copy(),
                    srcsl_c=src_sl.reshape(nch, P).T.copy(),
                    q1e_c=q1e.reshape(nch, P).T.copy(),
                    hvT=hvT, gid_sh=gid_sh, gid_li=gid_li)

    def bc(v):
        return np.broadcast_to(np.asarray(v, f32)[None, :], (P, DG)).copy()

    p = params
    w = {}
    w["pn_W_aug"] = np.concatenate([p["pn_W"], p["pn_b"][None, :]], 0).astype(f32)
    w["pe1_W_aug"] = np.concatenate([p["pe1_W"], p["pe1_b"][None, :]], 0).astype(f32)
    Wmsg1 = np.zeros((DG, 258), f32)
    Wmsg1[:, 0:DG] = p["et_W"]
    Wmsg1[:, DG] = p["pe2_W"][DG:2 * DG, 0]
    w["Wmsg1"] = Wmsg1
    w["et_b_bc"] = bc(p["et_b"])
    for gi, g in (("1", "gru1"), ("2", "gru2")):
        Wih = np.asarray(p[f"{g}_Wih"], f32)
        Whh = np.asarray(p[f"{g}_Whh"], f32)
        bih = np.asarray(p[f"{g}_bih"], f32) - Wih.sum(1)  # elu(+1) fold
        bhh = np.asarray(p[f"{g}_bhh"], f32)
        w[f"W{gi}ihT"] = Wih.T.copy()
        w[f"W{gi}hhT"] = Whh.T.copy()
        w[f"br{gi}"] = (bih[0:DG] + bhh[0:DG]).reshape(2, P).T.copy()
        w[f"bz{gi}"] = (bih[DG:2 * DG] + bhh[DG:2 * DG]).reshape(2, P).T.copy()
        w[f"bin{gi}"] = bih[2 * DG:3 * DG].reshape(2, P).T.copy()
        w[f"bhn{gi}"] = bhh[2 * DG:3 * DG].reshape(2, P).T.copy()
    w["qdW"] = p["pe_W"][0:DG, 0:1].astype(f32)
    w["qsW"] = p["pe_W"][DG:2 * DG, 0:1].astype(f32)
    w["pnodeW"] = np.asarray(p["pnode_W"], f32)
    w["pnode_b_bc"] = bc(p["pnode_b"])
    for t in range(2):
        w[f"roTop{t}"] = p["ro_logit_W"][t][0:DG, 0:1].astype(f32)
        w[f"roBot{t}"] = p["ro_logit_W"][t][DG:2 * DG, 0:1].astype(f32)
        w[f"roProjW{t}"] = np.asarray(p["ro_proj_W"][t], f32)
        w[f"roProjB{t}"] = bc(p["ro_proj_b"][t])
        Wih = np.asarray(p["ro_gru_Wih"][t], f32)
        w[f"roIhT{t}"] = Wih.T.copy()
        w[f"roHhT{t}"] = np.asarray(p["ro_gru_Whh"][t], f32).T.copy()
        w[f"roBi{t}"] = (np.asarray(p["ro_gru_bih"][t], f32) - Wih.sum(1))[None, :].copy()
        w[f"roBh{t}"] = np.asarray(p["ro_gru_bhh"][t], f32)[None, :].copy()

    scal = dict(q1b=float(p["pe2_b"][0]), qdb=float(p["pe_b"][0]),
                rob=[float(p["ro_logit_b"][t][0]) for t in range(2)])
    in_maps = []
    for c in range(NCORES):
        m = prep_core(c)
        m.update(w)
        in_maps.append(m)
    return in_maps, dict(NT=NT, K_CH=K_CH, N_SH=N_SH, EC=EC, GPC=GPC, **scal)


def _build(cfg, sim=False):
    import concourse.bass as bass
    import concourse.bacc as bacc
    import concourse.mybir as mybir
    import concourse.tile as tile
    from concourse.masks import make_identity

    F32 = mybir.dt.float32
    F32R = mybir.dt.float32r
    I32 = mybir.dt.int32
    AF = mybir.ActivationFunctionType
    OP = mybir.AluOpType

    NT, K_CH, N_SH = cfg["NT"], cfg["K_CH"], cfg["N_SH"]
    EC = cfg["EC"]
    NG = NT // 4

    nc = bacc.Bacc("TRN2", target_bir_lowering=False, debug=False, num_devices=NCORES)

    def din(name, shape, dt=None):
        return nc.dram_tensor(name, shape, dt or F32, kind="ExternalInput").ap()

    NCH = EC // P
    xeT_d = din("xeT", [81, EC], F32R)
    dstsh_d = din("dstsh_c", [P, NCH])
    dstli_d = din("dstli_c", [P, NCH], I32)
    srcsl_d = din("srcsl_c", [P, NCH], I32)
    q1e_d = din("q1e_c", [P, NCH])
    hvT_d = din("hvT", [65, N_SH], F32R)
    gid_sh_d = din("gid_sh", [N_SH, 1])
    gid_li_d = din("gid_li", [N_SH, 1], I32)
    pnW_d = din("pn_W_aug", [65, DG], F32R)
    pe1W_d = din("pe1_W_aug", [81, DG], F32R)
    Wmsg1_d = din("Wmsg1", [DG, 258], F32R)
    etb_d = din("et_b_bc", [P, DG])
    gwd = {}
    for gi in ("1", "2"):
        gwd[f"ihT{gi}"] = din(f"W{gi}ihT", [DG, 3 * DG], F32R)
        gwd[f"hhT{gi}"] = din(f"W{gi}hhT", [DG, 3 * DG], F32R)
        for b in ("br", "bz", "bin", "bhn"):
            gwd[f"{b}{gi}"] = din(f"{b}{gi}", [P, 2])
    qdW_d = din("qdW", [DG, 1], F32R)
    qsW_d = din("qsW", [DG, 1], F32R)
    pnodeW_d = din("pnodeW", [DG, DG], F32R)
    pnb_d = din("pnode_b_bc", [P, DG])
    rod = {}
    for t in range(2):
        rod[f"top{t}"] = din(f"roTop{t}", [DG, 1], F32R)
        rod[f"bot{t}"] = din(f"roBot{t}", [DG, 1], F32R)
        rod[f"projW{t}"] = din(f"roProjW{t}", [DG, DG], F32R)
        rod[f"projB{t}"] = din(f"roProjB{t}", [P, DG])
        rod[f"ihT{t}"] = din(f"roIhT{t}", [DG, 3 * DG], F32R)
        rod[f"hhT{t}"] = din(f"roHhT{t}", [DG, 3 * DG], F32R)
        rod[f"bi{t}"] = din(f"roBi{t}", [1, 3 * DG], F32R)
        rod[f"bh{t}"] = din(f"roBh{t}", [1, 3 * DG], F32R)

    out_d = nc.dram_tensor("o", [P, DG], F32, kind="ExternalOutput").ap()

    hvnT_dram = nc.dram_tensor("hvnT", [DG, N_SH], F32R).ap()
    h1T_dram = nc.dram_tensor("h1T", [DG, N_SH], F32R).ap()
    h2T_dram = nc.dram_tensor("h2T", [DG, N_SH], F32R).ap()
    ctxT_dram = nc.dram_tensor("ctxT", [DG, N_SH], F32R).ap()
    q1_dram = nc.dram_tensor("q1d", [N_SH, 1], F32).ap()
    qd_dram = nc.dram_tensor("qdd", [N_SH, 1], F32).ap()
    cc_in = nc.dram_tensor("cc_in", [N_SH, 258], F32).ap()
    cc_out = nc.dram_tensor("cc_out", [NCORES * N_SH, 258], F32,
                            addr_space="Shared").ap()

    with tile.TileContext(nc) as tc:
        _cms = []

        def enter_pool(**kw):
            cm = tc.tile_pool(**kw)
            _cms.append(cm)
            return cm, cm.__enter__()

        _, cp = enter_pool(name="consts", bufs=1)
        _, sp = enter_pool(name="w2", bufs=2)
        _, sp1 = enter_pool(name="w1", bufs=1)
        _, sp3 = enter_pool(name="w3", bufs=2)
        _, sp4 = enter_pool(name="w4", bufs=4)

        iota_f = cp.tile([P, P], F32)
        nc.gpsimd.iota(iota_f[:], pattern=[[1, P]], base=0, channel_multiplier=0,
                       allow_small_or_imprecise_dtypes=True)
        ident = cp.tile([P, P], F32)
        make_identity(nc, ident[:])
        identr = cp.tile([P, P], F32R)
        nc.vector.tensor_copy(out=identr[:], in_=ident[:])
        ones_f = cp.tile([1, P], F32)
        nc.vector.memset(ones_f[:], 1.0)
        ones_row = cp.tile([1, P], F32R)
        nc.vector.tensor_copy(out=ones_row[:], in_=ones_f[:])

        _ctr = [0]

        def _utag():
            _ctr[0] += 1
            return f"c{_ctr[0]}"

        def load(ap_d, shape, dt=F32, pool=None, tag=None):
            t = (pool or cp).tile(shape, dt, tag=tag or _utag())
            nc.sync.dma_start(out=t[:], in_=ap_d[:])
            return t

        def load2(ap_d, w, dt=F32R, pool=None, tags=None):
            out = []
            for h in range(2):
                t = (pool or cp).tile([P, w], dt, tag=tags[h] if tags else _utag())
                nc.sync.dma_start(out=t[:], in_=ap_d[h * P:(h + 1) * P, :])
                out.append(t)
            return out

        pnW = load(pnW_d, [65, DG], F32R)
        pe1W = load(pe1W_d, [81, DG], F32R)
        Wmsg1 = load2(Wmsg1_d, 258)
        etb = load(etb_d, [P, DG])
        qdW = load2(qdW_d, 1)
        qsW = load2(qsW_d, 1)
        pnodeW = load2(pnodeW_d, DG)
        pnb = load(pnb_d, [P, DG])
        GB = {}
        for gi in ("1", "2"):
            for b in ("br", "bz", "bin", "bhn"):
                GB[f"{b}{gi}"] = load(gwd[f"{b}{gi}"], [P, 2])

        # ---------- helpers ----------
        def safe_recip(s_ap):
            ind = sp4.tile([P, 1], F32, tag="ind")
            nc.vector.tensor_scalar(out=ind[:], in0=s_ap, scalar1=0.0,
                                    scalar2=None, op0=OP.is_gt)
            ssafe = sp4.tile([P, 1], F32, tag="ssafe")
            nc.vector.tensor_scalar(out=ssafe[:], in0=ind[:], scalar1=-1.0,
                                    scalar2=1.0, op0=OP.mult, op1=OP.add)
            nc.vector.tensor_tensor(out=ssafe[:], in0=ssafe[:], in1=s_ap, op=OP.add)
            inv = sp4.tile([P, 1], F32, tag="inv")
            nc.vector.reciprocal(out=inv[:], in_=ssafe[:])
            return ind, inv

        def elu1(src_ap, out_tag):
            mn = sp3.tile([P, DG], F32, tag="mn")
            nc.vector.tensor_scalar(out=mn[:], in0=src_ap, scalar1=0.0, scalar2=None,
                                    op0=OP.min)
            ex = sp3.tile([P, DG], F32, tag="ex")
            nc.scalar.activation(out=ex[:], in_=mn[:], func=AF.Exp)
            rl = sp3.tile([P, DG], F32, tag=out_tag)
            nc.scalar.activation(out=rl[:], in_=src_ap, func=AF.Relu)
            nc.vector.tensor_tensor(out=rl[:], in0=rl[:], in1=ex[:], op=OP.add)
            return rl

        def seg_post(ctx_ps, ppt, bias_bc, nt):
            ind, inv = safe_recip(ctx_ps[:, 256:257])
            cn = sp3.tile([P, DG], F32, tag="cn")
            nc.vector.tensor_scalar(out=cn[:], in0=ctx_ps[:, 0:DG], scalar1=inv[:, :1],
                                    scalar2=None, op0=OP.mult)
            tb = sp3.tile([P, DG], F32, tag="tb")
            nc.vector.tensor_scalar(out=tb[:], in0=bias_bc[:], scalar1=ind[:, :1],
                                    scalar2=None, op0=OP.mult)
            nc.vector.tensor_tensor(out=cn[:], in0=cn[:], in1=tb[:], op=OP.add)
            rl = elu1(cn[:], "rlp")
            for h in range(2):
                tp = ppt.tile([P, P], F32, tag="tpost")
                nc.tensor.transpose(out=tp[:], in_=rl[:, h * P:(h + 1) * P],
                                    identity=ident[:])
                cc = sp4.tile([P, P], F32R, tag="ctxc")
                nc.vector.tensor_copy(out=cc[:], in_=tp[:])
                nc.sync.dma_start(out=ctxT_dram[h * P:(h + 1) * P, nt * P:(nt + 1) * P],
                                  in_=cc[:])

        def edge_phase(layer, pp2, pp1):
            for nt in range(NT):
                ctx_ps = pp2.tile([P, 258], F32, tag="ctx_ps")
                nch = 0
                rem = K_CH
                while rem > 0:
                    take = min(4, rem)
                    base = (nt * K_CH + nch) * P
                    W = take * P
                    if layer == 1:
                        xe = sp.tile([81, 512], F32R, tag="xe")
                        nc.sync.dma_start(out=xe[:, 0:W], in_=xeT_d[:, base:base + W])
                        he_ps, he = [], []
                        for h in range(2):
                            hp = pp1.tile([P, 512], F32, tag=f"he{h}")
                            nc.tensor.matmul(out=hp[:, 0:W],
                                             lhsT=pe1W[:, h * P:(h + 1) * P],
                                             rhs=xe[:, 0:W], start=True, stop=True)
                            hs = sp.tile([P, 512], F32R, tag=f"hes{h}")
                            nc.scalar.activation(out=hs[:, 0:W], in_=hp[:, 0:W],
                                                 func=AF.Prelu, alpha=0.01)
                            he_ps.append(hp)
                            he.append(hs)
                    cb = nt * K_CH + nch
                    dstf4 = sp4.tile([P, 4], F32, tag="dstf")
                    nc.sync.dma_start(out=dstf4[:, 0:take], in_=dstsh_d[:, cb:cb + take])
                    if layer == 1:
                        q1e4 = sp4.tile([P, 4], F32, tag="q1e")
                        nc.sync.dma_start(out=q1e4[:, 0:take], in_=q1e_d[:, cb:cb + take])
                    else:
                        dsti4 = sp4.tile([P, 4], I32, tag="dsti")
                        nc.sync.dma_start(out=dsti4[:, 0:take], in_=dstli_d[:, cb:cb + take])
                        srci4 = sp4.tile([P, 4], I32, tag="srci")
                        nc.sync.dma_start(out=srci4[:, 0:take], in_=srcsl_d[:, cb:cb + take])
                    for s in range(take):
                        eb = base + s * P
                        if layer == 1:
                            qg = q1e4[:, s:s + 1]
                            mg_ps = pp2.tile([P, 258], F32, tag="mg_ps")
                            for h in range(2):
                                nc.tensor.matmul(out=mg_ps[:],
                                                 lhsT=he[h][:, s * P:(s + 1) * P],
                                                 rhs=Wmsg1[h][:], start=(h == 0),
                                                 stop=(h == 1))
                            logit_src = mg_ps[:, 256:257]
                            val_src = mg_ps[:, 0:DG]
                        else:
                            gath = sp4.tile([P, 258], F32, tag="gath")
                            nc.gpsimd.indirect_dma_start(
                                out=gath[:], out_offset=None, in_=cc_out[:],
                                in_offset=bass.IndirectOffsetOnAxis(ap=srci4[:, s:s + 1], axis=0))
                            qgt = sp4.tile([P, 1], F32, tag="qg")
                            nc.gpsimd.indirect_dma_start(
                                out=qgt[:], out_offset=None, in_=qd_dram[:],
                                in_offset=bass.IndirectOffsetOnAxis(ap=dsti4[:, s:s + 1], axis=0))
                            qg = qgt
                            logit_src = gath[:, 256:257]
                            val_src = gath[:, 0:DG]
                        lr = sp4.tile([P, 1], F32, tag="lr")
                        nc.scalar.activation(out=lr[:], in_=logit_src, func=AF.Prelu,
                                             bias=qg[:, 0:1], alpha=0.01)
                        msg = sp4.tile([P, 258], F32R, tag="msg")
                        nc.scalar.activation(out=msg[:, 256:257], in_=lr[:], func=AF.Exp)
                        nc.vector.tensor_scalar(out=msg[:, 0:DG], in0=val_src,
                                                scalar1=msg[:, 256:257].bitcast(F32), scalar2=None,
                                                op0=OP.mult)
                        nc.vector.tensor_scalar(out=msg[:, 257:258], in0=lr[:],
                                                scalar1=0.0, scalar2=None, op0=OP.mult)
                        oh = sp4.tile([P, P], F32R, tag="oh")
                        nc.vector.tensor_scalar(out=oh[:], in0=iota_f[:],
                                                scalar1=dstf4[:, s:s + 1], scalar2=None,
                                                op0=OP.is_equal)
                        nc.tensor.matmul(out=ctx_ps[:], lhsT=oh[:], rhs=msg[:],
                                         start=(nch == 0 and s == 0),
                                         stop=(rem == take and s == take - 1))
                    nch += take
                    rem -= take
                seg_post(ctx_ps, pp1, etb if layer == 1 else pnb, nt)

        def gru_phase(gi, h_dram, ppg, ppm, post_group):
            """GRU over 512-node groups; x from ctxT_dram, h from h_dram.
            post_group(g, h_out_tiles) emits per-group extras."""
            ihT = load2(gwd[f"ihT{gi}"], 3 * DG, pool=sp1, tags=("wihA", "wihB"))
            hhT = load2(gwd[f"hhT{gi}"], 3 * DG, pool=sp1, tags=("whhA", "whhB"))
            br, bz = GB[f"br{gi}"], GB[f"bz{gi}"]
            bin_, bhn = GB[f"bin{gi}"], GB[f"bhn{gi}"]
            for g in range(NG):
                c0 = g * 512
                hv = []
                for h in range(2):
                    t = sp.tile([P, 512], F32R, tag=f"hv{h}")
                    nc.sync.dma_start(out=t[:], in_=h_dram[h * P:(h + 1) * P, c0:c0 + 512])
                    hv.append(t)
                x = []
                for h in range(2):
                    t = sp.tile([P, 512], F32R, tag=f"xc{h}")
                    nc.sync.dma_start(out=t[:], in_=ctxT_dram[h * P:(h + 1) * P, c0:c0 + 512])
                    x.append(t)
                hout = []
                for jh in range(2):
                    def mk(jofs, sides, tag):
                        ps = ppg.tile([P, 512], F32, tag=tag)
                        n = 0
                        tot = 2 * len(sides)
                        for (W2, rhs2) in sides:
                            for k in range(2):
                                nc.tensor.matmul(
                                    out=ps[:],
                                    lhsT=W2[k][:, jofs + jh * P:jofs + (jh + 1) * P],
                                    rhs=rhs2[k][:], start=(n == 0), stop=(n == tot - 1))
                                n += 1
                        return ps
                    p_r = mk(0, [(ihT, x), (hhT, hv)], "gps_r")
                    p_z = mk(DG, [(ihT, x), (hhT, hv)], "gps_z")
                    p_i = mk(2 * DG, [(ihT, x)], "gps_i")
                    p_h = mk(2 * DG, [(hhT, hv)], "gps_h")
                    r = sp3.tile([P, 512], F32, tag="r")
                    nc.scalar.activation(out=r[:], in_=p_r[:], func=AF.Sigmoid,
                                         bias=br[:, jh:jh + 1])
                    z = sp3.tile([P, 512], F32, tag="z")
                    nc.scalar.activation(out=z[:], in_=p_z[:], func=AF.Sigmoid,
                                         bias=bz[:, jh:jh + 1])
                    t1 = sp3.tile([P, 512], F32, tag="t1")
                    nc.vector.tensor_scalar(out=t1[:], in0=p_h[:], scalar1=bhn[:, jh:jh + 1],
                                            scalar2=None, op0=OP.add)
                    nc.vector.tensor_tensor(out=t1[:], in0=r[:], in1=t1[:], op=OP.mult)
                    nc.vector.tensor_tensor(out=t1[:], in0=p_i[:], in1=t1[:], op=OP.add)
                    nn = sp3.tile([P, 512], F32, tag="nn")
                    nc.scalar.activation(out=nn[:], in_=t1[:], func=AF.Tanh,
                                         bias=bin_[:, jh:jh + 1])
                    d = sp3.tile([P, 512], F32, tag="d")
                    nc.vector.tensor_tensor(out=d[:], in0=hv[jh][:], in1=nn[:],
                                            op=OP.subtract)
                    nc.vector.tensor_tensor(out=d[:], in0=z[:], in1=d[:], op=OP.mult)
                    nc.vector.tensor_tensor(out=d[:], in0=nn[:], in1=d[:], op=OP.add)
                    ho = sp.tile([P, 512], F32R, tag=f"ho{jh}")
                    nc.scalar.activation(out=ho[:], in_=d[:], func=AF.Relu)
                    hout.append(ho)
                post_group(g, hout, ppm)

        # ---------- Phase A ----------
        ppA_cm, ppA = enter_pool(name="ppA", bufs=2, space="PSUM")
        for g in range(NG):
            c0 = g * 512
            hvt = sp.tile([65, 512], F32R, tag="hvt")
            nc.sync.dma_start(out=hvt[:], in_=hvT_d[:, c0:c0 + 512])
            hvn = []
            for h in range(2):
                ps = ppA.tile([P, 512], F32, tag="Aps")
                nc.tensor.matmul(out=ps[:], lhsT=pnW[:, h * P:(h + 1) * P], rhs=hvt[:],
                                 start=True, stop=True)
                hm = sp.tile([P, 512], F32R, tag=f"hvn{h}")
                nc.scalar.activation(out=hm[:], in_=ps[:], func=AF.Prelu, alpha=0.01)
                nc.sync.dma_start(out=hvnT_dram[h * P:(h + 1) * P, c0:c0 + 512], in_=hm[:])
                hvn.append(hm)
        ppA_cm.__exit__(None, None, None)

        # ---------- Phase B ----------
        pB2_cm, pB2 = enter_pool(name="pB2", bufs=2, space="PSUM")
        pB1_cm, pB1 = enter_pool(name="pB1", bufs=1, space="PSUM")
        edge_phase(1, pB2, pB1)
        pB1_cm.__exit__(None, None, None)
        pB2_cm.__exit__(None, None, None)

        # ---------- Phase C: GRU1 -> h1, pre_msg, qd, qs ----------
        pC1_cm, pC1 = enter_pool(name="pC1", bufs=1, space="PSUM")
        pC2_cm, pC2 = enter_pool(name="pC2", bufs=2, space="PSUM")

        def c_post(g, hout, ppm):
            c0 = g * 512
            for jh in range(2):
                nc.sync.dma_start(out=h1T_dram[jh * P:(jh + 1) * P, c0:c0 + 512],
                                  in_=hout[jh][:])
            for s in range(4):
                pm = ppm.tile([P, DG], F32, tag="pm")
                for k in range(2):
                    nc.tensor.matmul(out=pm[:], lhsT=hout[k][:, s * P:(s + 1) * P],
                                     rhs=pnodeW[k][:], start=(k == 0), stop=(k == 1))
                pms = sp4.tile([P, DG], F32, tag="pms")
                nc.vector.tensor_copy(out=pms[:], in_=pm[:])
                nc.sync.dma_start(out=cc_in[c0 + s * P:c0 + (s + 1) * P, 0:DG], in_=pms[:])
            for (Wv, dstap, bias, tg) in (
                    (qdW, qd_dram[c0:c0 + 512, 0:1], cfg["qdb"], "qrow"),
                    (qsW, cc_in[c0:c0 + 512, 256:257], 0.0, "qrow")):
                qp = ppm.tile([1, 512], F32, tag=tg)
                for k in range(2):
                    nc.tensor.matmul(out=qp[:], lhsT=Wv[k][:], rhs=hout[k][:],
                                     start=(k == 0), stop=(k == 1))
                qs = sp4.tile([1, 512], F32, tag="qrs")
                nc.scalar.activation(out=qs[:], in_=qp[:], func=AF.Copy, bias=bias)
                nc.sync.dma_start(out=dstap.rearrange("a b -> b a"), in_=qs[:])

        gru_phase("1", hvnT_dram, pC1, pC2, c_post)
        pC2_cm.__exit__(None, None, None)
        pC1_cm.__exit__(None, None, None)

        # ---------- Phase D: AllGather ----------
        if sim:
            # timeline-sim stand-in: local copy approximating collective cost
            for cc in range(NCORES):
                nc.sync.dma_start(out=cc_out[cc * N_SH:(cc + 1) * N_SH, :],
                                  in_=cc_in[:])
        else:
            nc.gpsimd.collective_compute(
                "AllGather", mybir.AluOpType.bypass,
                replica_groups=[list(range(NCORES))],
                ins=[cc_in[:]], outs=[cc_out[:]])

        # ---------- Phase E ----------
        pE2_cm, pE2 = enter_pool(name="pE2", bufs=2, space="PSUM")
        pE1_cm, pE1 = enter_pool(name="pE1", bufs=1, space="PSUM")
        edge_phase(2, pE2, pE1)
        pE1_cm.__exit__(None, None, None)
        pE2_cm.__exit__(None, None, None)

        # ---------- Phase F: GRU2 -> h2, g0 ----------
        pF1_cm, pF1 = enter_pool(name="pF1", bufs=1, space="PSUM")
        pF2_cm, pF2 = enter_pool(name="pF2", bufs=2, space="PSUM")
        g0_ps = pF1.tile([P, DG], F32, tag="g0ps")
        nmm = [0]

        def f_post(g, hout, ppm):
            c0 = g * 512
            for jh in range(2):
                nc.sync.dma_start(out=h2T_dram[jh * P:(jh + 1) * P, c0:c0 + 512],
                                  in_=hout[jh][:])
            for s in range(4):
                nt = g * 4 + s
                h2n = sp4.tile([P, DG], F32R, tag="h2n")
                for h in range(2):
                    tp = ppm.tile([P, P], F32R, tag="ftp")
                    nc.tensor.transpose(out=tp[:], in_=hout[h][:, s * P:(s + 1) * P],
                                        identity=identr[:])
                    nc.vector.tensor_copy(out=h2n[:, h * P:(h + 1) * P], in_=tp[:])
                gidf = sp4.tile([P, 1], F32, tag="gidf")
                nc.sync.dma_start(out=gidf[:], in_=gid_sh_d[nt * P:(nt + 1) * P, :])
                ohg = sp4.tile([P, P], F32R, tag="ohg")
                nc.vector.tensor_scalar(out=ohg[:], in0=iota_f[:], scalar1=gidf[:, :1],
                                        scalar2=None, op0=OP.is_equal)
                nc.tensor.matmul(out=g0_ps[:], lhsT=ohg[:], rhs=h2n[:],
                                 start=(nmm[0] == 0), stop=(nmm[0] == NT - 1))
                nmm[0] += 1

        gru_phase("2", h1T_dram, pF1, pF2, f_post)
        g0sb = cp.tile([P, DG], F32)
        nc.vector.tensor_copy(out=g0sb[:], in_=g0_ps[:])
        pF2_cm.__exit__(None, None, None)
        pF1_cm.__exit__(None, None, None)

        # ---------- Phase G: readout ----------
        pG1_cm, pG1 = enter_pool(name="pG1", bufs=1, space="PSUM")
        pG2_cm, pG2 = enter_pool(name="pG2", bufs=2, space="PSUM")

        def transpose2(src_ap_fn, dst_tiles):
            for h in range(2):
                tp = pG1.tile([P, P], F32, tag="gtp1")
                nc.tensor.transpose(out=tp[:], in_=src_ap_fn(h), identity=ident[:])
                nc.vector.tensor_copy(out=dst_tiles[h][:], in_=tp[:])

        g_cur = g0sb
        for t in range(2):
            roTop = load2(rod[f"top{t}"], 1, pool=sp1, tags=("rotA", "rotB"))
            roBot = load2(rod[f"bot{t}"], 1, pool=sp1, tags=("robA", "robB"))
            roPW = load2(rod[f"projW{t}"], DG, pool=sp1, tags=("ropA", "ropB"))
            roPB = load(rod[f"projB{t}"], [P, DG], pool=sp1, tag="ropb")
            roIh = load2(rod[f"ihT{t}"], 3 * DG, pool=sp1, tags=("wihA", "wihB"))
            roHh = load2(rod[f"hhT{t}"], 3 * DG, pool=sp1, tags=("whhA", "whhB"))
            roBi = load(rod[f"bi{t}"], [1, 3 * DG], F32R, pool=sp1, tag="robi")
            roBh = load(rod[f"bh{t}"], [1, 3 * DG], F32R, pool=sp1, tag="robh")

            rg = sp3.tile([P, DG], F32, tag="rg")
            nc.scalar.activation(out=rg[:], in_=g_cur[:], func=AF.Relu)
            rgT = [sp4.tile([P, P], F32R, tag=f"rgT{h}", name=f"rgT{h}") for h in range(2)]
            transpose2(lambda h: rg[:, h * P:(h + 1) * P], rgT)
            qgp = pG1.tile([1, P], F32, tag="smallp")
            for h in range(2):
                nc.tensor.matmul(out=qgp[:], lhsT=roTop[h][:], rhs=rgT[h][:],
                                 start=(h == 0), stop=(h == 1))
            qgs = sp4.tile([1, P], F32, tag="qgs")
            nc.scalar.activation(out=qgs[:], in_=qgp[:], func=AF.Copy, bias=cfg["rob"][t])
            nc.sync.dma_start(out=q1_dram[0:P, 0:1].rearrange("a b -> b a"), in_=qgs[:])

            wg_ps = pG1.tile([P, 258], F32, tag="wgps")
            for g in range(NG):
                c0 = g * 512
                h2g = []
                for h in range(2):
                    tt = sp.tile([P, 512], F32R, tag=f"hv{h}")
                    nc.sync.dma_start(out=tt[:], in_=h2T_dram[h * P:(h + 1) * P, c0:c0 + 512])
                    h2g.append(tt)
                qhp = pG1.tile([1, 512], F32, tag="smallp")
                for h in range(2):
                    nc.tensor.matmul(out=qhp[:], lhsT=roBot[h][:], rhs=h2g[h][:],
                                     start=(h == 0), stop=(h == 1))
                qhs = sp4.tile([1, 512], F32, tag="qhs")
                nc.vector.tensor_copy(out=qhs[:], in_=qhp[:])
                for s in range(4):
                    nt = g * 4 + s
                    qtp = pG1.tile([P, 1], F32, tag="qtp")
                    nc.tensor.transpose(out=qtp[:], in_=qhs[0:1, s * P:(s + 1) * P],
                                        identity=ident[0:1, 0:1])
                    gidi = sp4.tile([P, 1], I32, tag="gidi")
                    nc.sync.dma_start(out=gidi[:], in_=gid_li_d[nt * P:(nt + 1) * P, :])
                    qgg = sp4.tile([P, 1], F32, tag="qgg")
                    nc.gpsimd.indirect_dma_start(
                        out=qgg[:], out_offset=None, in_=q1_dram[:],
                        in_offset=bass.IndirectOffsetOnAxis(ap=gidi[:, :1], axis=0))
                    lr = sp4.tile([P, 1], F32, tag="lr")
                    nc.scalar.activation(out=lr[:], in_=qtp[:], func=AF.Prelu,
                                         bias=qgg[:, :1], alpha=0.01)
                    h2n = sp4.tile([P, DG], F32, tag="h2nG")
                    for h in range(2):
                        tp = pG2.tile([P, P], F32R, tag="gtpr")
                        nc.tensor.transpose(out=tp[:], in_=h2g[h][:, s * P:(s + 1) * P],
                                            identity=identr[:])
                        nc.vector.tensor_copy(out=h2n[:, h * P:(h + 1) * P], in_=tp[:])
                    hw = sp4.tile([P, 258], F32R, tag="hw")
                    nc.scalar.activation(out=hw[:, 256:257], in_=lr[:], func=AF.Exp)
                    nc.vector.tensor_scalar(out=hw[:, 0:DG], in0=h2n[:],
                                            scalar1=hw[:, 256:257].bitcast(F32), scalar2=None,
                                            op0=OP.mult)
                    nc.vector.tensor_scalar(out=hw[:, 257:258], in0=lr[:],
                                            scalar1=0.0, scalar2=None, op0=OP.mult)
                    gidf = sp4.tile([P, 1], F32, tag="gidf")
                    nc.sync.dma_start(out=gidf[:], in_=gid_sh_d[nt * P:(nt + 1) * P, :])
                    ohg = sp4.tile([P, P], F32R, tag="ohg")
                    nc.vector.tensor_scalar(out=ohg[:], in0=iota_f[:], scalar1=gidf[:, :1],
                                            scalar2=None, op0=OP.is_equal)
                    nc.tensor.matmul(out=wg_ps[:], lhsT=ohg[:], rhs=hw[:],
                                     start=(nt == 0), stop=(nt == NT - 1))
            ind, inv = safe_recip(wg_ps[:, 256:257])
            wgn = sp3.tile([P, DG], F32, tag="wgn")
            nc.vector.tensor_scalar(out=wgn[:], in0=wg_ps[:, 0:DG], scalar1=inv[:, :1],
                                    scalar2=None, op0=OP.mult)
            wgnT = [sp4.tile([P, P], F32R, tag=f"wgnT{h}", name=f"wgnT{h}") for h in range(2)]
            transpose2(lambda h: wgn[:, h * P:(h + 1) * P], wgnT)
            prj = pG1.tile([P, DG], F32, tag="smallp")
            for h in range(2):
                nc.tensor.matmul(out=prj[:], lhsT=wgnT[h][:], rhs=roPW[h][:],
                                 start=(h == 0), stop=(h == 1))
            tb = sp3.tile([P, DG], F32, tag="tb")
            nc.vector.tensor_scalar(out=tb[:], in0=roPB[:], scalar1=ind[:, :1],
                                    scalar2=None, op0=OP.mult)
            grp = sp3.tile([P, DG], F32, tag="grp")
            nc.vector.tensor_tensor(out=grp[:], in0=prj[:], in1=tb[:], op=OP.add)
            cfed = elu1(grp[:], "cfed")
            cT = [sp4.tile([P, P], F32R, tag=f"cT{h}", name=f"cT{h}") for h in range(2)]
            gT = [sp4.tile([P, P], F32R, tag=f"gT{h}", name=f"gT{h}") for h in range(2)]
            transpose2(lambda h: cfed[:, h * P:(h + 1) * P], cT)
            transpose2(lambda h: g_cur[:, h * P:(h + 1) * P], gT)

            def ro_gate(jofs, xT, WT, brow, tag):
                ps = pG2.tile([P, DG], F32, tag=tag)
                for k in range(2):
                    nc.tensor.matmul(out=ps[:], lhsT=xT[k][:],
                                     rhs=WT[k][:, jofs:jofs + DG],
                                     start=(k == 0), stop=False)
                nc.tensor.matmul(out=ps[:], lhsT=ones_row[:],
                                 rhs=brow[:, jofs:jofs + DG], start=False, stop=True)
                return ps

            ph_sb = {}
            for (nm, jofs) in (("r", 0), ("z", DG), ("n", 2 * DG)):
                ps = ro_gate(jofs, gT, roHh, roBh, "rops")
                sb = sp3.tile([P, DG], F32, tag=f"ph{nm}")
                nc.vector.tensor_copy(out=sb[:], in_=ps[:])
                ph_sb[nm] = sb
            r = sp3.tile([P, DG], F32, tag="ror")
            ps = ro_gate(0, cT, roIh, roBi, "rops")
            nc.vector.tensor_tensor(out=r[:], in0=ps[:], in1=ph_sb["r"][:], op=OP.add)
            nc.scalar.activation(out=r[:], in_=r[:], func=AF.Sigmoid)
            z = sp3.tile([P, DG], F32, tag="roz")
            ps = ro_gate(DG, cT, roIh, roBi, "rops")
            nc.vector.tensor_tensor(out=z[:], in0=ps[:], in1=ph_sb["z"][:], op=OP.add)
            nc.scalar.activation(out=z[:], in_=z[:], func=AF.Sigmoid)
            ps = ro_gate(2 * DG, cT, roIh, roBi, "rops")
            nc.vector.tensor_tensor(out=ph_sb["n"][:], in0=r[:], in1=ph_sb["n"][:],
                                    op=OP.mult)
            nc.vector.tensor_tensor(out=ph_sb["n"][:], in0=ps[:], in1=ph_sb["n"][:],
                                    op=OP.add)
            nn = sp3.tile([P, DG], F32, tag="ronn")
            nc.scalar.activation(out=nn[:], in_=ph_sb["n"][:], func=AF.Tanh)
            gnew = cp.tile([P, DG], F32, tag=f"gnew{t}")
            nc.vector.tensor_tensor(out=gnew[:], in0=g_cur[:], in1=nn[:], op=OP.subtract)
            nc.vector.tensor_tensor(out=gnew[:], in0=z[:], in1=gnew[:], op=OP.mult)
            nc.vector.tensor_tensor(out=gnew[:], in0=nn[:], in1=gnew[:], op=OP.add)
            g_cur = gnew
        nc.sync.dma_start(out=out_d[:], in_=g_cur[:])
        pG2_cm.__exit__(None, None, None)
        pG1_cm.__exit__(None, None, None)

        for cm in reversed(_cms[:5]):
            cm.__exit__(None, None, None)

    nc.compile()
    return nc


def kernel(node_feats, edge_feats, src, dst, graph_ids, params):
    from concourse import bass_utils
    node_feats = np.asarray(node_feats, np.float32)
    edge_feats = np.asarray(edge_feats, np.float32)
    src = np.asarray(src).astype(np.int64)
    dst = np.asarray(dst).astype(np.int64)
    graph_ids = np.asarray(graph_ids).astype(np.int64)
    params = {k: np.asarray(v) for k, v in params.items()}

    in_maps, cfg = _host_prep(node_feats, edge_feats, src, dst, graph_ids, params)
    nc = _build(cfg)
    res = bass_utils.run_bass_kernel_spmd(nc, in_maps, core_ids=list(range(NCORES)))
    out = np.concatenate([res.results[c]["o"][:cfg["GPC"]] for c in range(NCORES)], 0)
    return out.astype(np.float32)
